# revision 10
# baseline (speedup 1.0000x reference)
"""Trainium2 Bass kernel for nn_ContextualAttention.

Per sample b (one per NeuronCore):
    X   = foreground[b]               # [256, 4096]  (channels x pixels)
    K   = (X + eps).T, L2-normalized rows          # [4096, 256]
    S   = K @ X                        # [4096(k), 4096(p)] scores
    A   = softmax(S, axis=k)
    out = K.T @ A                      # [256, 4096]

On-chip layout (per core):
    X_sb    [128, 2, HW]   channels on partitions (2 chunks of 128), f32r
    Khat    [128, KT, 256] k on partitions (KT tiles of 128), channels free
    scores tile [128(k), 512(p)] in PSUM <- mm over c (2 accum steps)
    E = exp(recip_n[k] * s)  via ACT with per-partition scale, PSUM->SBUF
    out_psum [128(c), 512(p)] += Khat_tile.T @ E   (accum over k tiles)
    Z[1, 512] += ones.T @ E                        (softmax denominator)
    out = out_psum * bcast(1/Z)  (broadcast via ones-outer-product matmul)

Matmuls run in float32r (full-rate fp32 on the PE); every tile feeding a
matmul is produced with dtype float32r to satisfy walrus's rounding check.

eps=1e-7 is dropped: its effect on the output is O(1e-7) relative, far
below matmul precision.
"""

import numpy as np
from contextlib import ExitStack

import concourse.bass as bass
import concourse.tile as tile
from concourse import mybir
from concourse.bass_utils import run_bass_kernel_spmd
from concourse.masks import make_identity

F32 = mybir.dt.float32
F32R = mybir.dt.float32r
AF = mybir.ActivationFunctionType
ALU = mybir.AluOpType

CH = 256     # channels
P = 128      # partitions
PT = 512     # pixel-tile width (matmul moving dim / psum bank)
N_CORES = 8


def _emit(tc: "tile.TileContext", x: bass.AP, out: bass.AP, hw: int):
    nc = tc.nc
    CC = CH // P          # channel chunks (2)
    KT = hw // P          # k tiles (32)
    NPT = hw // PT        # pixel tiles (8)

    with ExitStack() as ctx:
        const = ctx.enter_context(tc.tile_pool(name="const", bufs=1))
        sb = ctx.enter_context(tc.tile_pool(name="sb", bufs=1))

        X = sb.tile([P, CC, hw], F32R, tag="X")
        XT = sb.tile([P, KT, CH], F32, tag="XT")
        Khat = sb.tile([P, KT, CH], F32R, tag="Khat")
        n2 = sb.tile([P, KT], F32, tag="n2")
        recip_n = sb.tile([P, KT], F32, tag="recip_n")

        ident = const.tile([P, P], F32, tag="ident")
        ones_f = const.tile([P, 1], F32, tag="ones_f")
        ones_rf = const.tile([1, P], F32, tag="ones_rf")
        ones_col = const.tile([P, 1], F32R, tag="ones_col")
        ones_row = const.tile([1, P], F32R, tag="ones_row")
        make_identity(nc, ident)
        nc.vector.memset(ones_f, 1.0)
        nc.vector.memset(ones_rf, 1.0)
        with nc.allow_low_precision(reason="f32r matmul operand prep"):
            nc.vector.tensor_copy(ones_col, ones_f)
            nc.vector.tensor_copy(ones_row, ones_rf)

        # ---- load X: [256, hw] -> [128, cc, hw] ----
        NQ = 4
        qs = hw // NQ
        for cc in range(CC):
            for q in range(NQ):
                nc.sync.dma_start(
                    out=X[:, cc, q * qs:(q + 1) * qs],
                    in_=x[cc * P:(cc + 1) * P, q * qs:(q + 1) * qs].bitcast(F32R),
                )

        # ---- setup: transpose X -> XT; n2 = row sumsq; Khat = XT/|XT| ----
        with tc.tile_pool(name="tpsum", bufs=2, space="PSUM") as tpsum, \
             tc.tile_pool(name="tsq", bufs=2) as tsq:
            for kt in range(KT):
                pt_ = tpsum.tile([P, CH], F32, tag="t")
                for cc in range(CC):
                    nc.tensor.transpose(
                        pt_[:, cc * P:(cc + 1) * P],
                        X[:, cc, kt * P:(kt + 1) * P].bitcast(F32),
                        ident,
                    )
                nc.scalar.copy(XT[:, kt, :], pt_)
                sq = tsq.tile([P, CH], F32, tag="sq")
                nc.scalar.activation(
                    sq,
                    XT[:, kt, :],
                    AF.Square,
                    accum_out=n2[:, kt:kt + 1],
                )
            # recip_n = 1/sqrt(n2)
            nc.scalar.sqrt(n2, n2)
            nc.vector.reciprocal(recip_n, n2)
            # Khat = XT * recip_n (per-partition scalar), rounded to f32r
            with nc.allow_low_precision(reason="f32r matmul operand prep"):
                for kt in range(KT):
                    nc.vector.tensor_scalar_mul(
                        out=Khat[:, kt, :],
                        in0=XT[:, kt, :],
                        scalar1=recip_n[:, kt:kt + 1],
                    )

        # ---- main: per pixel-tile flash attention ----
        with tc.tile_pool(name="ps", bufs=2, space="PSUM") as ps_pool, \
             tc.tile_pool(name="acc", bufs=6, space="PSUM") as acc_pool, \
             tc.tile_pool(name="ework", bufs=4) as e_pool, \
             tc.tile_pool(name="owork", bufs=4) as o_pool, \
             tc.tile_pool(name="zwork", bufs=2) as z_pool:
            for pt in range(NPT):
                out_ps = [
                    acc_pool.tile([P, PT], F32, tag="acc", name=f"out_ps{cc}")
                    for cc in range(CC)
                ]
                z_ps = acc_pool.tile([1, PT], F32, tag="acc")
                for kc in range(KT):
                    # scores[k, p] = sum_c X[c, k] * X[c, p]
                    s_ps = ps_pool.tile([P, PT], F32, tag="ps")
                    for cc in range(CC):
                        nc.tensor.matmul(
                            s_ps,
                            lhsT=X[:, cc, kc * P:(kc + 1) * P],
                            rhs=X[:, cc, pt * PT:(pt + 1) * PT],
                            start=(cc == 0),
                            stop=(cc == CC - 1),
                        )
                    # E = exp(recip_n[k] * s)
                    e_sb = e_pool.tile([P, PT], F32R, tag="e")
                    nc.scalar.activation(
                        e_sb, s_ps, AF.Exp, scale=recip_n[:, kc:kc + 1],
                    )
                    # out[c, p] += Khat[k, c].T @ E ; Z[p] += ones.T @ E
                    for cc in range(CC):
                        nc.tensor.matmul(
                            out_ps[cc],
                            lhsT=Khat[:, kc, cc * P:(cc + 1) * P],
                            rhs=e_sb,
                            start=(kc == 0),
                            stop=(kc == KT - 1),
                        )
                    nc.tensor.matmul(
                        z_ps,
                        lhsT=ones_col,
                        rhs=e_sb,
                        start=(kc == 0),
                        stop=(kc == KT - 1),
                    )
                # normalize: out = out_ps * (1/Z) broadcast along partitions
                rz_sb = z_pool.tile([1, PT], F32R, tag="rz")
                with nc.allow_low_precision(reason="per-column scale, not an accum"):
                    nc.vector.reciprocal(rz_sb, z_ps)
                bc_ps = ps_pool.tile([P, PT], F32, tag="ps")
                nc.tensor.matmul(
                    bc_ps,
                    lhsT=ones_row,
                    rhs=rz_sb,
                    start=True,
                    stop=True,
                )
                bc_sb = z_pool.tile([P, PT], F32, tag="bc")
                nc.scalar.copy(bc_sb, bc_ps)
                for cc in range(CC):
                    o_sb = o_pool.tile([P, PT], F32, tag="o")
                    nc.vector.tensor_mul(o_sb, out_ps[cc], bc_sb)
                    nc.sync.dma_start(
                        out=out[cc * P:(cc + 1) * P, pt * PT:(pt + 1) * PT],
                        in_=o_sb,
                    )


def _legalize_single_wait(nc: bass.Bass) -> None:
    """The walrus build in this container accepts at most ONE sync-wait per
    instruction ("Too many sync wait commands"); Tile emits instructions with
    one wait per outstanding producer. Hoist extra waits onto injected
    same-engine NOPs placed immediately before the instruction — identical
    blocking semantics, one wait each."""
    for fn in nc.m.functions:
        for bb in fn.blocks:
            new = []
            changed = False
            for inst in bb.instructions:
                if (
                    isinstance(inst, mybir.InstISA)
                    and inst.engine == mybir.EngineType.Pool
                ):
                    # Tail-of-kernel semaphore RANGE_CLEAR on GpSimd; this
                    # walrus build rejects its encoding ("ISA wrong length").
                    # Semaphores are re-initialized by the runtime at
                    # execution start, so the in-kernel clear is redundant.
                    # (DVE InstISA ops — e.g. tensor_tensor_reduce — are real
                    # compute and must be kept.)
                    changed = True
                    continue
                si = inst.sync_info
                if si is not None and si.on_wait is not None and len(si.on_wait) > 1:
                    waits = list(si.on_wait)
                    for j, w in enumerate(waits[:-1]):
                        nop = mybir.InstNoOp(
                            name=f"{inst.name}-xw{j}",
                            engine=inst.engine,
                            sync_info=mybir.SyncInfo(on_wait=[w], on_update=[]),
                            bass_nofuse=True,
                        )
                        new.append(nop)
                    si.on_wait = [waits[-1]]
                    changed = True
                new.append(inst)
            if changed:
                bb.instructions = new


def build_nc(hw: int = 4096, legalize: bool = True) -> bass.Bass:
    nc = bass.Bass()
    x = nc.dram_tensor("x", [CH, hw], F32, kind="ExternalInput")
    out = nc.dram_tensor("out", [CH, hw], F32, kind="ExternalOutput")
    with tile.TileContext(nc) as tc:
        _emit(tc, x[:], out[:], hw)
    if legalize:
        _legalize_single_wait(nc)
    return nc


_nc_cache: dict = {}


def kernel(foreground: np.ndarray) -> np.ndarray:
    fg = np.ascontiguousarray(np.asarray(foreground, dtype=np.float32))
    bs, ch, h, w = fg.shape
    assert bs == N_CORES and ch == CH
    hw = h * w
    if hw not in _nc_cache:
        _nc_cache[hw] = build_nc(hw)
    nc = _nc_cache[hw]
    in_maps = [{"x": fg[i].reshape(ch, hw)} for i in range(bs)]
    res = run_bass_kernel_spmd(nc, in_maps, core_ids=list(range(bs)))
    return np.stack(
        [np.asarray(res.results[i]["out"]).reshape(ch, h, w) for i in range(bs)]
    )


# revision 11
# speedup vs baseline: 1.2249x; 1.2249x over previous
"""Trainium2 Bass kernel for nn_ContextualAttention.

Per sample b (one per NeuronCore):
    X   = foreground[b]               # [256, 4096]  (channels x pixels)
    K   = (X + eps).T, L2-normalized rows          # [4096, 256]
    S   = K @ X                        # [4096(k), 4096(p)] scores
    A   = softmax(S, axis=k)
    out = K.T @ A                      # [256, 4096]

On-chip layout (per core):
    X_sb    [128, 2, HW]   channels on partitions (2 chunks of 128), f32r
    Khat    [128, KT, 256] k on partitions (KT tiles of 128), channels free
    scores tile [128(k), 512(p)] in PSUM <- mm over c (2 accum steps)
    E = exp(recip_n[k] * s)  via ACT with per-partition scale, PSUM->SBUF
    out_psum [128(c), 512(p)] += Khat_tile.T @ E   (accum over k tiles)
    Z[1, 512] += ones.T @ E                        (softmax denominator)
    out = out_psum * bcast(1/Z)  (broadcast via ones-outer-product matmul)

Matmuls run in float32r (full-rate fp32 on the PE); every tile feeding a
matmul is produced with dtype float32r to satisfy walrus's rounding check.

eps=1e-7 is dropped: its effect on the output is O(1e-7) relative, far
below matmul precision.
"""

import numpy as np
from contextlib import ExitStack

import concourse.bass as bass
import concourse.tile as tile
from concourse import mybir
from concourse.bass_utils import run_bass_kernel_spmd
from concourse.masks import make_identity

F32 = mybir.dt.float32
F32R = mybir.dt.float32r
AF = mybir.ActivationFunctionType
ALU = mybir.AluOpType

CH = 256     # channels
P = 128      # partitions
PT = 512     # pixel-tile width (matmul moving dim / psum bank)
N_CORES = 8


def _emit(tc: "tile.TileContext", x: bass.AP, out: bass.AP, hw: int):
    nc = tc.nc
    CC = CH // P          # channel chunks (2)
    KT = hw // P          # k tiles (32)
    NPT = hw // PT        # pixel tiles (8)

    with ExitStack() as ctx:
        const = ctx.enter_context(tc.tile_pool(name="const", bufs=1))
        sb = ctx.enter_context(tc.tile_pool(name="sb", bufs=1))

        X = sb.tile([P, CC, hw], F32R, tag="X")
        XT = sb.tile([P, KT, CH], F32, tag="XT")
        Khat = sb.tile([P, KT, CH], F32R, tag="Khat")
        n2 = sb.tile([P, KT], F32, tag="n2")
        recip_n = sb.tile([P, KT], F32, tag="recip_n")

        ident = const.tile([P, P], F32, tag="ident")
        ones_f = const.tile([P, P], F32, tag="ones_f")
        ones128 = const.tile([P, P], F32R, tag="ones128")
        make_identity(nc, ident)
        nc.vector.memset(ones_f, 1.0)
        with nc.allow_low_precision(reason="f32r matmul operand prep"):
            nc.vector.tensor_copy(ones128, ones_f)

        # ---- load X: [256, hw] -> [128, cc, hw] ----
        NQ = 4
        qs = hw // NQ
        for cc in range(CC):
            for q in range(NQ):
                nc.sync.dma_start(
                    out=X[:, cc, q * qs:(q + 1) * qs],
                    in_=x[cc * P:(cc + 1) * P, q * qs:(q + 1) * qs].bitcast(F32R),
                )

        # ---- setup: transpose X -> XT; n2 = row sumsq; Khat = XT/|XT| ----
        with tc.tile_pool(name="tpsum", bufs=2, space="PSUM") as tpsum, \
             tc.tile_pool(name="tsq", bufs=2) as tsq:
            for kt in range(KT):
                pt_ = tpsum.tile([P, CH], F32, tag="t")
                for cc in range(CC):
                    nc.tensor.transpose(
                        pt_[:, cc * P:(cc + 1) * P],
                        X[:, cc, kt * P:(kt + 1) * P].bitcast(F32),
                        ident,
                    )
                nc.scalar.copy(XT[:, kt, :], pt_)
                sq = tsq.tile([P, CH], F32, tag="sq")
                nc.scalar.activation(
                    sq,
                    XT[:, kt, :],
                    AF.Square,
                    accum_out=n2[:, kt:kt + 1],
                )
            # recip_n = 1/sqrt(n2)
            nc.scalar.sqrt(n2, n2)
            nc.vector.reciprocal(recip_n, n2)
            # Khat = XT * recip_n (per-partition scalar), rounded to f32r
            with nc.allow_low_precision(reason="f32r matmul operand prep"):
                for kt in range(KT):
                    nc.vector.tensor_scalar_mul(
                        out=Khat[:, kt, :],
                        in0=XT[:, kt, :],
                        scalar1=recip_n[:, kt:kt + 1],
                    )

        # ---- main: per pixel-tile flash attention ----
        with tc.tile_pool(name="ps", bufs=3, space="PSUM") as ps_pool, \
             tc.tile_pool(name="acc", bufs=4, space="PSUM") as acc_pool, \
             tc.tile_pool(name="zps", bufs=1, space="PSUM") as zps_pool, \
             tc.tile_pool(name="ework", bufs=6) as e_pool, \
             tc.tile_pool(name="owork", bufs=4) as o_pool, \
             tc.tile_pool(name="zwork", bufs=2) as z_pool:
            for pt in range(NPT):
                out_ps = [
                    acc_pool.tile([P, PT], F32, tag="acc", name=f"out_ps{cc}")
                    for cc in range(CC)
                ]
                z_ps = zps_pool.tile([P, PT], F32, tag="z")
                for kc in range(KT):
                    # scores[k, p] = sum_c X[c, k] * X[c, p]
                    s_ps = ps_pool.tile([P, PT], F32, tag="ps")
                    for cc in range(CC):
                        nc.tensor.matmul(
                            s_ps,
                            lhsT=X[:, cc, kc * P:(kc + 1) * P],
                            rhs=X[:, cc, pt * PT:(pt + 1) * PT],
                            start=(cc == 0),
                            stop=(cc == CC - 1),
                        )
                    # E = exp(recip_n[k] * s)
                    e_sb = e_pool.tile([P, PT], F32R, tag="e")
                    nc.scalar.activation(
                        e_sb, s_ps, AF.Exp, scale=recip_n[:, kc:kc + 1],
                    )
                    # out[c, p] += Khat[k, c].T @ E ; Z[p] += ones.T @ E
                    for cc in range(CC):
                        nc.tensor.matmul(
                            out_ps[cc],
                            lhsT=Khat[:, kc, cc * P:(cc + 1) * P],
                            rhs=e_sb,
                            start=(kc == 0),
                            stop=(kc == KT - 1),
                        )
                    nc.tensor.matmul(
                        z_ps,
                        lhsT=ones128,
                        rhs=e_sb,
                        start=(kc == 0),
                        stop=(kc == KT - 1),
                    )
                # normalize: Z arrives replicated on all partitions, so
                # 1/Z is a full-width DVE op and no broadcast is needed.
                rz_sb = z_pool.tile([P, PT], F32, tag="rz")
                nc.vector.reciprocal(rz_sb, z_ps)
                for cc in range(CC):
                    o_sb = o_pool.tile([P, PT], F32, tag="o")
                    nc.vector.tensor_mul(o_sb, out_ps[cc], rz_sb)
                    nc.sync.dma_start(
                        out=out[cc * P:(cc + 1) * P, pt * PT:(pt + 1) * PT],
                        in_=o_sb,
                    )


def _legalize_single_wait(nc: bass.Bass) -> None:
    """The walrus build in this container accepts at most ONE sync-wait per
    instruction ("Too many sync wait commands"); Tile emits instructions with
    one wait per outstanding producer. Hoist extra waits onto injected
    same-engine NOPs placed immediately before the instruction — identical
    blocking semantics, one wait each."""
    for fn in nc.m.functions:
        for bb in fn.blocks:
            new = []
            changed = False
            for inst in bb.instructions:
                if (
                    isinstance(inst, mybir.InstISA)
                    and inst.engine == mybir.EngineType.Pool
                ):
                    # Tail-of-kernel semaphore RANGE_CLEAR on GpSimd; this
                    # walrus build rejects its encoding ("ISA wrong length").
                    # Semaphores are re-initialized by the runtime at
                    # execution start, so the in-kernel clear is redundant.
                    # (DVE InstISA ops — e.g. tensor_tensor_reduce — are real
                    # compute and must be kept.)
                    changed = True
                    continue
                si = inst.sync_info
                if si is not None and si.on_wait is not None and len(si.on_wait) > 1:
                    waits = list(si.on_wait)
                    for j, w in enumerate(waits[:-1]):
                        nop = mybir.InstNoOp(
                            name=f"{inst.name}-xw{j}",
                            engine=inst.engine,
                            sync_info=mybir.SyncInfo(on_wait=[w], on_update=[]),
                            bass_nofuse=True,
                        )
                        new.append(nop)
                    si.on_wait = [waits[-1]]
                    changed = True
                new.append(inst)
            if changed:
                bb.instructions = new


def build_nc(hw: int = 4096, legalize: bool = True) -> bass.Bass:
    nc = bass.Bass()
    x = nc.dram_tensor("x", [CH, hw], F32, kind="ExternalInput")
    out = nc.dram_tensor("out", [CH, hw], F32, kind="ExternalOutput")
    with tile.TileContext(nc) as tc:
        _emit(tc, x[:], out[:], hw)
    if legalize:
        _legalize_single_wait(nc)
    return nc


_nc_cache: dict = {}


def kernel(foreground: np.ndarray) -> np.ndarray:
    fg = np.ascontiguousarray(np.asarray(foreground, dtype=np.float32))
    bs, ch, h, w = fg.shape
    assert bs == N_CORES and ch == CH
    hw = h * w
    if hw not in _nc_cache:
        _nc_cache[hw] = build_nc(hw)
    nc = _nc_cache[hw]
    in_maps = [{"x": fg[i].reshape(ch, hw)} for i in range(bs)]
    res = run_bass_kernel_spmd(nc, in_maps, core_ids=list(range(bs)))
    return np.stack(
        [np.asarray(res.results[i]["out"]).reshape(ch, h, w) for i in range(bs)]
    )
